# revision 26
# baseline (speedup 1.0000x reference)
"""Trainium2 Bass kernel for nn_DualWeightAttention (B=2, S=2048, H=2048, 16 heads).

Sharding: tensor-parallel over heads — 2 heads per core on 8 cores.
Each core computes q/k/v projections for its 2 heads, attention for those
heads (both batches), and a partial output projection against its 256-row
slice of Wo.T. The 8 partial [4096, 2048] fp16 outputs are summed on the host
in f32.

v2 schedule: the projection phase and the attention phase are MERGED so the
PE never starves while ScalarE does the exp stream:
  - Phase A projects batch 0 only (~82us of PE work, baseline structure).
  - The 16 attention periods then interleave, per period:
      QK(i+1) kt-pair matmuls -> exp -> mask-mult   (ACT/DVE paced)
      PV(i) accumulation                            (PE)
      batch-1 projection groups (periods 0-7, one 16-matmul group per
      2 kt-pairs, single shared PSUM bank)          (PE filler)
      output-projection tiles drawn from a BACKLOG queue paced so late
      periods (no projection filler left) stay PE-bound (PE filler)
  - Softmax denominator: DVE tree folds the attn slab to [128(k), 512(q)],
    then ONE GpSimd partition_all_reduce produces the broadcast row-sums
    [128, 512] directly (replaces the baseline's ones-matmul + reciprocal +
    partition_broadcast chain and frees a PSUM bank). Launched at j==3 of
    the unit's own period so its ~3us latency hides before uT_mult needs it.
  - Outputs are written as fp16 partials (halves output DMA; host sums in
    f32; adds ~3e-4 rel error, budget is 2e-2).

PSUM budget (8 banks): QK pairs 2x[P,2,QC]=4, PV accumulators 2, outproj 1,
b1-projection shared bank 1.
"""

import numpy as np

import concourse.mybir as mybir
import concourse.tile as tile
from concourse import bacc
from concourse import bass_isa
from concourse.bass_utils import run_bass_kernel_spmd

P = 128
B = 2
S = 2048
H = 2048
NH = 16
HD = 128
NCORES = 8
HPC = NH // NCORES  # heads per core
DC = HPC * HD       # d-columns per core
QC = 512            # q-chunk (matmul moving free dim)
HT = H // P         # contraction tiles for projections
SCALE = 1.0 / float(np.sqrt(HD))

F32 = mybir.dt.float32
BF16 = mybir.dt.float16  # fp16 over bf16: same PE/DVE rates, finer mantissa

PROJ_DT = BF16  # hsT + wq/wk/wv
QK_DT = BF16    # qT/kT operands
OUT_DT = BF16   # uT + woT
MASK_DT = BF16
EXP = mybir.ActivationFunctionType.Exp
ADD = mybir.AluOpType.add
MULT = mybir.AluOpType.mult


def build_attention_nc(s=S):
    bs = B * s
    kt_n = s // P     # k tiles per batch
    kp_n = kt_n // 2  # kt pairs
    nq = s // QC      # q chunks per batch
    st_n = s // P     # s tiles per batch (out projection)
    vt_n = bs // P    # v tiles (both batches)
    KH = kt_n // 2    # kt per mask half
    NQT = 4
    KOQ = HT // NQT   # hsT streamed as 4 quarter-K tiles per s-chunk
    NOUT = (QC // P) * (H // QC)  # outproj tiles per chunk (16)
    NCH = B * nq      # total chunks (8)

    nc = bacc.Bacc("TRN2", target_bir_lowering=False, debug=False, num_devices=NCORES)
    hsT = nc.dram_tensor("hsT", [H, bs], PROJ_DT, kind="ExternalInput")
    maskT = nc.dram_tensor("maskT", [B, s, s], MASK_DT, kind="ExternalInput")
    wqT = nc.dram_tensor("wqT", [H, DC], PROJ_DT, kind="ExternalInput")
    wkT = nc.dram_tensor("wkT", [H, DC], PROJ_DT, kind="ExternalInput")
    wvT = nc.dram_tensor("wvT", [H, DC], PROJ_DT, kind="ExternalInput")
    woT = nc.dram_tensor("woT", [DC, H], OUT_DT, kind="ExternalInput")
    out = nc.dram_tensor("out", [bs, H], OUT_DT, kind="ExternalOutput")

    hsT_r = hsT.ap().rearrange("(o p) t -> p o t", p=P)
    wq_r = wqT.ap().rearrange("(o p) d -> p o d", p=P)
    wk_r = wkT.ap().rearrange("(o p) d -> p o d", p=P)
    wv_r = wvT.ap().rearrange("(o p) d -> p o d", p=P)
    wo_r = woT.ap().rearrange("(h p) j -> p h j", p=P)
    out_r = out.ap().rearrange("(t p) j -> p t j", p=P)

    with tile.TileContext(nc) as tc:
        with (
            tc.tile_pool(name="persist", bufs=1) as persist,
        ):
            qT = persist.tile([P, HPC, bs], QK_DT)
            kT = persist.tile([P, HPC, bs], QK_DT)
            vsb = persist.tile([P, vt_n, DC], BF16)
            wo_sb = persist.tile([P, HPC, H], OUT_DT)
            wq_sb = persist.tile([P, HT, DC], PROJ_DT, name="wq_sb")
            wk_sb = persist.tile([P, HT, DC], PROJ_DT, name="wk_sb")
            wv_sb = persist.tile([P, HT, DC], PROJ_DT, name="wv_sb")

            # evacuation helper: alternate DVE/ACT so neither paces the PE,
            # with an optional forced engine for load balancing
            _ev = [0]

            def evac(dst, src, eng=None):
                if eng is None:
                    eng = "s" if _ev[0] % 2 == 0 else "v"
                    _ev[0] += 1
                if eng == "s":
                    nc.scalar.copy(dst, src)
                else:
                    nc.vector.tensor_copy(dst, src)

            # hsT quarter tiles stream through a ring shared by both phases
            hpool_cm = tc.tile_pool(name="hpool", bufs=6)
            hpool = hpool_cm.__enter__()
            quarters = {}  # sc -> [4 quarter tiles]

            def hst_issue(sc, eng=None):
                eng = eng or nc.sync
                ssl = slice(sc * QC, (sc + 1) * QC)
                qs = []
                for qf in range(NQT):
                    hst = hpool.tile([P, KOQ, QC], PROJ_DT, tag="hst", name="hst")
                    eng.dma_start(
                        hst[:], hsT_r[:, qf * KOQ : (qf + 1) * KOQ, ssl]
                    )
                    qs.append(hst)
                quarters[sc] = qs

            def hq(sc, ko):
                return quarters[sc][ko // KOQ][:, ko % KOQ]

            # mask halves: ring of 3, issued one per period start
            mpool_cm = tc.tile_pool(name="mpool", bufs=3)
            mpool = mpool_cm.__enter__()
            mhalves = {}  # (chunk, mh) -> tile

            def mask_issue(ch, mh):
                if (ch, mh) in mhalves:
                    return
                b, qq = divmod(ch, nq)
                ms = mpool.tile([P, KH, QC], MASK_DT, tag="mslab", name="ms")
                nc.sync.dma_start(
                    ms[:],
                    maskT.ap()[b].rearrange("(kt p) q -> p kt q", p=P)[
                        :, mh * KH : (mh + 1) * KH,
                        qq * QC : (qq + 1) * QC,
                    ],
                )
                mhalves[(ch, mh)] = ms

            # ---------------- Phase A: batch-0 projections ----------------
            with (
                tc.tile_pool(name="ppsum", bufs=2, space="PSUM") as ppsum,
                tc.tile_pool(name="vpsum", bufs=4, space="PSUM") as vpsum,
            ):
                # DMA order: first q-projection group needs wq quarter 0 and
                # the first hsT quarter — issue those first.
                # issue the critical first tiles on TWO DGE queues in
                # parallel: wq quarter 0 on Sync, hsT chunk 0 on ScalarE
                # (idle at startup); the rest follows on Sync
                nc.sync.dma_start(wq_sb[:, 0:4], wq_r[:, 0:4])
                hst_issue(0, nc.scalar)
                for _wf in range(1, 4):
                    _wsl = slice(_wf * (HT // 4), (_wf + 1) * (HT // 4))
                    nc.sync.dma_start(wq_sb[:, _wsl], wq_r[:, _wsl])
                nc.sync.dma_start(wk_sb[:], wk_r)
                nc.sync.dma_start(wv_sb[:], wv_r)
                nc.sync.dma_start(wo_sb[:], wo_r)

                for sc in range(nq):  # batch 0 chunks only
                    if sc > 0:
                        hst_issue(sc)
                    ssl = slice(sc * QC, (sc + 1) * QC)
                    if sc == 2:
                        mask_issue(0, 0)
                        mask_issue(0, 1)
                    # q(h0), q(h1), k(h0), k(h1) — q first so the wk DMA has
                    # more slack at startup
                    for wsb, dstT in ((wq_sb, qT), (wk_sb, kT)):
                        for h in range(HPC):
                            ps = ppsum.tile([P, QC], F32, tag="psqk")
                            for ko in range(HT):
                                nc.tensor.matmul(
                                    ps[:],
                                    wsb[:, ko, h * P : (h + 1) * P],
                                    hq(sc, ko),
                                    start=(ko == 0),
                                    stop=(ko == HT - 1),
                                )
                            evac(dstT[:, h, ssl], ps[:])
                    # v: ko-outer over 4 concurrent PSUM groups
                    psvs = []
                    for st in range(QC // P):
                        psv = vpsum.tile([P, DC], F32, tag="psv")
                        psvs.append(psv)
                    for ko in range(HT):
                        for st in range(QC // P):
                            nc.tensor.matmul(
                                psvs[st][:],
                                hq(sc, ko)[:, st * P : (st + 1) * P],
                                wv_sb[:, ko, :],
                                start=(ko == 0),
                                stop=(ko == HT - 1),
                            )
                    for st in range(QC // P):
                        evac(vsb[:, sc * (QC // P) + st, :], psvs[st][:])
                # prefetch first b1 chunk's hsT for the phase-2 filler
                hst_issue(nq)

            # ---------------- Phase 2: merged attention + b1 projections ----------------
            with (
                tc.tile_pool(name="apool", bufs=2) as apool,
                tc.tile_pool(name="tpool", bufs=2) as tpool,
                tc.tile_pool(name="upool", bufs=2) as upool,
                tc.tile_pool(name="rpool", bufs=1) as rpool,
                tc.tile_pool(name="opool", bufs=6) as opool,
                tc.tile_pool(name="spsum", bufs=2, space="PSUM") as spsum,
                tc.tile_pool(name="upsum", bufs=2, space="PSUM") as upsum,
            ):
                # single-bank pools for periods 0-7 (outproj tiles + b1
                # projections), manually scoped: both release after period 7
                # and their two banks recycle as ONE two-bank outproj pool so
                # late-period tiles are drawn in adjacent-jc PAIRS with a
                # single [128,1024] evac + single DMA per pair
                opsum_cm = tc.tile_pool(name="opsum", bufs=1, space="PSUM")
                opsum = opsum_cm.__enter__()
                pproj_cm = tc.tile_pool(name="pproj", bufs=1, space="PSUM")
                pproj = pproj_cm.__enter__()
                odsum_holder = [None, None]  # (cm, pool)
                units = [
                    (b, qq, h)
                    for b in range(B)
                    for qq in range(nq)
                    for h in range(HPC)
                ]
                nu = len(units)
                aslabs = {}
                psus = {}
                s8s = {}
                rsums = {}
                rbcs = {}
                uTs = {}

                def qk_pair(i, j):
                    # two scoresT k-tile matmuls into one 2-bank psum tile;
                    # exp(s+m) = exp(s)*exp(m): ScalarE exp evacuates the
                    # [128,1024] pair in one ACT op; the host-precomputed
                    # exp(mask) factor is one fp16 [128,1024] DVE multiply
                    b, qq, h = units[i]
                    ch = i // HPC
                    if j == 0:
                        asl = apool.tile([P, kt_n, QC], BF16, tag="aslab", name="asl")
                        aslabs[i] = asl
                    asl = aslabs[i]
                    pss = spsum.tile([P, 2, QC], F32, tag="pss")
                    for u in range(2):
                        kt = 2 * j + u
                        nc.tensor.matmul(
                            pss[:, u],
                            kT[:, h, b * s + kt * P : b * s + (kt + 1) * P],
                            qT[:, h, b * s + qq * QC : b * s + (qq + 1) * QC],
                            start=True,
                            stop=True,
                        )
                    nc.scalar.activation(asl[:, 2 * j : 2 * j + 2], pss[:], EXP)
                    # mask multiply batched over TWO pairs ([128, 2048] per
                    # DVE op, 4 ops/unit instead of 8 — amortizes op
                    # overhead; 4-slab groups align exactly with mask halves)
                    if j % 2 == 1:
                        ms = mhalves[(ch, (2 * j) // KH)]
                        mo = (2 * j - 2) % KH
                        nc.vector.tensor_tensor(
                            asl[:, 2 * j - 2 : 2 * j + 2],
                            asl[:, 2 * j - 2 : 2 * j + 2],
                            ms[:, mo : mo + 4],
                            MULT,
                        )

                def pv_part(i, kt):
                    b, qq, h = units[i]
                    asl = aslabs[i]
                    if kt == 0:
                        psu = upsum.tile([P, QC], F32, tag="psu")
                        psus[i] = psu
                    nc.tensor.matmul(
                        psus[i][:],
                        vsb[:, b * kt_n + kt, h * P : (h + 1) * P],
                        asl[:, kt],
                        start=(kt == 0),
                        stop=(kt == kt_n - 1),
                    )

                def den_tree_level(i, lvl):
                    # kt-fold of the attn slab into a scratch tile (asl is
                    # fully written at period start); fp16 2x DVE adds
                    asl = aslabs[i]
                    if lvl == 0:
                        s8 = tpool.tile([P, KH, QC], BF16, tag="s8", name="s8")
                        s8s[i] = s8
                        nc.vector.tensor_tensor(
                            s8[:], asl[:, 0:KH], asl[:, KH : 2 * KH], ADD
                        )
                    else:
                        s8 = s8s[i]
                        w = KH >> lvl
                        nc.vector.tensor_tensor(
                            s8[:, 0:w], s8[:, 0:w], s8[:, w : 2 * w], ADD
                        )

                def den_all_reduce(i):
                    # ONE GpSimd partition_all_reduce turns the folded
                    # [128(k), 512(q)] tile into broadcast row sums [128, 512]
                    rsum = rpool.tile([P, QC], F32, tag="rsum", name="rsum")
                    nc.gpsimd.partition_all_reduce(
                        rsum[:], s8s.pop(i)[:, 0], 128, bass_isa.ReduceOp.add
                    )
                    rsums[i] = rsum

                def den_recip(i):
                    rbc = rpool.tile([P, QC], F32, tag="rbc", name="rbc")
                    nc.vector.reciprocal_approx_fast(out=rbc[:], in_=rsums.pop(i)[:])
                    rbcs[i] = rbc

                def uT_mult(i):
                    # normalize + evacuate the PV accumulator in one DVE op
                    b, qq, h = units[i]
                    if b not in uTs:
                        uT_new = upool.tile([P, HPC, s], OUT_DT, tag="uT", name="uT")
                        uTs[b] = uT_new
                    nc.vector.tensor_tensor(
                        uTs[b][:, h, qq * QC : (qq + 1) * QC],
                        psus.pop(i)[:],
                        rbcs.pop(i)[:],
                        MULT,
                    )

                def outproj_tile(b, qq, t, pso=None, eng=None):
                    # one output-projection tile: 2 accumulating matmuls
                    # (h=0,1) + evac + DMA out
                    st_l, jc = divmod(t, H // QC)
                    st = qq * (QC // P) + st_l
                    uT_b = uTs[b]
                    if pso is None:
                        pso = opsum.tile([P, QC], F32, tag="pso", name="pso")
                    for h in range(HPC):
                        nc.tensor.matmul(
                            pso[:],
                            uT_b[:, h, st * P : (st + 1) * P],
                            wo_sb[:, h, jc * QC : (jc + 1) * QC],
                            start=(h == 0),
                            stop=(h == HPC - 1),
                        )
                    ot = opool.tile([P, QC], OUT_DT, tag="ot", name="ot", bufs=3)
                    evac(ot[:], pso[:], eng=eng)
                    nc.sync.dma_start(
                        out_r[:, b * st_n + st, jc * QC : (jc + 1) * QC], ot[:]
                    )

                def outproj_pair(b, qq, t0, pso2=None, eng=None):
                    # two adjacent-jc tiles into a 2-bank psum tile: 4 MMs,
                    # ONE [128,1024] evac, ONE contiguous DMA
                    st_l, jc = divmod(t0, H // QC)
                    st = qq * (QC // P) + st_l
                    uT_b = uTs[b]
                    if pso2 is None:
                        pso2 = odsum_holder[1].tile(
                            [P, 2, QC], F32, tag="psd", name="psd"
                        )
                    for u in range(2):
                        for h in range(HPC):
                            nc.tensor.matmul(
                                pso2[:, u],
                                uT_b[:, h, st * P : (st + 1) * P],
                                wo_sb[:, h, (jc + u) * QC : (jc + u + 1) * QC],
                                start=(h == 0),
                                stop=(h == HPC - 1),
                            )
                    ot = opool.tile([P, 2, QC], OUT_DT, tag="ot2", name="ot2", bufs=3)
                    evac(ot[:], pso2[:], eng=eng)
                    nc.sync.dma_start(
                        out_r[:, b * st_n + st, jc * QC : (jc + 2) * QC], ot[:]
                    )

                # ---- b1 projection filler groups (shared single PSUM bank)
                def proj_group_qk(wsb, dstT, h, sc):
                    ps = pproj.tile([P, QC], F32, tag="pj", name="pj")
                    ssl = slice(sc * QC, (sc + 1) * QC)
                    for ko in range(HT):
                        nc.tensor.matmul(
                            ps[:],
                            wsb[:, ko, h * P : (h + 1) * P],
                            hq(sc, ko),
                            start=(ko == 0),
                            stop=(ko == HT - 1),
                        )
                    evac(dstT[:, h, ssl], ps[:])

                def proj_group_v(st, sc):
                    psv = pproj.tile([P, DC], F32, tag="pj", name="pj")
                    for ko in range(HT):
                        nc.tensor.matmul(
                            psv[:],
                            hq(sc, ko)[:, st * P : (st + 1) * P],
                            wv_sb[:, ko, :],
                            start=(ko == 0),
                            stop=(ko == HT - 1),
                        )
                    evac(vsb[:, sc * (QC // P) + st, :], psv[:])

                proj_jobs = []
                for sc in range(nq, 2 * nq):  # b1 global chunks 4..7
                    proj_jobs.append(("qk", wq_sb, qT, 0, sc))
                    proj_jobs.append(("qk", wq_sb, qT, 1, sc))
                    proj_jobs.append(("qk", wk_sb, kT, 0, sc))
                    proj_jobs.append(("qk", wk_sb, kT, 1, sc))
                    proj_jobs.append(("v", None, None, 0, sc))
                    proj_jobs.append(("v", None, None, 1, sc))
                    proj_jobs.append(("v", None, None, 2, sc))
                    proj_jobs.append(("v", None, None, 3, sc))
                _pj = [0]

                def emit_proj_job():
                    if _pj[0] >= len(proj_jobs):
                        return
                    kind, wsb, dstT, x, sc = proj_jobs[_pj[0]]
                    _pj[0] += 1
                    if kind == "qk":
                        proj_group_qk(wsb, dstT, x, sc)
                    else:
                        proj_group_v(x, sc)
                    # prefetch the next b1 chunk's hsT once this chunk's last
                    # group is emitted
                    if _pj[0] % 8 == 0 and sc + 1 < 2 * nq:
                        hst_issue(sc + 1)

                # outproj backlog pacing: uniform 8/period (availability
                # capped: chunk c's 16 tiles appear at period 2c+2). Early
                # periods draw singles at even j (projection groups own odd
                # j); late periods draw PAIRS at odd j.
                DRAW = [0, 0] + [8] * 14
                JD_EARLY = [0, 0, 2, 0, 2, 0, 2, 2]
                JD_LATE = [0, 2, 0, 2, 0, 2, 0, 2]
                backlog = []
                # outproj evac engine pattern: split between the engines
                EVPAT = ("s", "v")

                # ---- pipeline prologue: unit 0's QK pairs
                for j in range(kp_n):
                    qk_pair(0, j)

                for i in range(nu):
                    b, qq, h = units[i]
                    if i == 8:
                        # projections done: recycle the pproj + opsum banks
                        # as one two-bank pool for pair-drawn outproj tiles
                        pproj_cm.__exit__(None, None, None)
                        opsum_cm.__exit__(None, None, None)
                        cm = tc.tile_pool(name="odsum", bufs=1, space="PSUM")
                        odsum_holder[0] = cm
                        odsum_holder[1] = cm.__enter__()
                    # mask prefetch: chunk i//2+1, half i%2
                    ch_next = i // 2 + 1
                    if ch_next < NCH:
                        mask_issue(ch_next, i % 2)
                    # denominator chain of i-1 + uT evac (rbc long ready)
                    if i >= 1:
                        den_recip(i - 1)
                        uT_mult(i - 1)
                        if units[i - 1][2] == HPC - 1:
                            pc = (i - 1) // 2
                            pb, pqq = pc // nq, pc % nq
                            backlog.extend((pb, pqq, t) for t in range(NOUT))
                    jdist = JD_EARLY if i < 8 else JD_LATE
                    ndraw = 0
                    for j in range(kp_n):
                        if i + 1 < nu:
                            qk_pair(i + 1, j)
                        pv_part(i, 2 * j)
                        pv_part(i, 2 * j + 1)
                        if j <= 3:
                            den_tree_level(i, j)
                        if j == 3:
                            den_all_reduce(i)
                        if i < 8:
                            for _ in range(jdist[j]):
                                if backlog and ndraw < DRAW[i]:
                                    pb, pqq, t = backlog.pop(0)
                                    outproj_tile(
                                        pb, pqq, t, eng=EVPAT[ndraw % len(EVPAT)]
                                    )
                                    ndraw += 1
                        elif jdist[j] and len(backlog) >= 2 and ndraw < DRAW[i]:
                            pb, pqq, t = backlog.pop(0)
                            backlog.pop(0)
                            outproj_pair(
                                pb, pqq, t, eng=EVPAT[(ndraw // 2) % len(EVPAT)]
                            )
                            ndraw += 2
                        if i < 8 and j % 2 == 1:
                            emit_proj_job()
                # drain: last unit's den chain + final chunk's outproj with
                # deep psum pipelining (borrow the freed QK pair banks)
                den_recip(nu - 1)
                uT_mult(nu - 1)
                rest = list(backlog) + [(B - 1, nq - 1, t) for t in range(NOUT)]
                odsum = odsum_holder[1]
                for n in range(0, len(rest), 2):
                    ob, oqq, t0 = rest[n]
                    sl = (n // 2) % 3
                    if sl < 2:
                        pso2 = spsum.tile([P, 2, QC], F32, tag="pss", name="big")
                    else:
                        pso2 = odsum.tile([P, 2, QC], F32, tag="psd", name="psd")
                    outproj_pair(ob, oqq, t0, pso2=pso2)
                odsum_holder[0].__exit__(None, None, None)

            mpool_cm.__exit__(None, None, None)
            hpool_cm.__exit__(None, None, None)
    nc.compile()
    return nc


def make_in_maps(hs, mask, Wq, Wk, Wv, Wo):
    """Host-side prep: transpose/shard the full inputs into per-core maps."""
    bs = hs.shape[0] * hs.shape[1]
    proj_np = np.float16
    out_np = np.float16
    hsT = np.ascontiguousarray(hs.reshape(bs, H).T).astype(proj_np)
    maskT = np.exp(
        np.ascontiguousarray(mask[:, 0].transpose(0, 2, 1))
    ).astype(np.float16)
    in_maps = []
    for c in range(NCORES):
        sl = slice(c * DC, (c + 1) * DC)
        in_maps.append(
            {
                "hsT": hsT,
                "maskT": maskT,
                "wqT": np.ascontiguousarray((Wq[sl] * SCALE).T).astype(proj_np),
                "wkT": np.ascontiguousarray(Wk[sl].T).astype(proj_np),
                "wvT": np.ascontiguousarray(Wv[sl].T).astype(proj_np),
                "woT": np.ascontiguousarray(Wo[:, sl].T).astype(out_np),
            }
        )
    return in_maps


_NC_CACHE = {}


def get_nc(s=S):
    if s not in _NC_CACHE:
        _NC_CACHE[s] = build_attention_nc(s)
    return _NC_CACHE[s]


def run(hs, mask, Wq, Wk, Wv, Wo, trace=False, trace_kwargs=None):
    s = hs.shape[1]
    nc = get_nc(s)
    in_maps = make_in_maps(hs, mask, Wq, Wk, Wv, Wo)
    res = run_bass_kernel_spmd(
        nc,
        in_maps,
        core_ids=list(range(NCORES)),
        trace=trace,
        **(trace_kwargs or {}),
    )
    parts = np.stack([r["out"] for r in res.results])
    full = parts.astype(np.float32).sum(axis=0)
    return full.reshape(hs.shape[0], s, H), res


def kernel(hidden_states, attention_mask, Wq, Wk, Wv, Wo):
    hs = np.asarray(hidden_states, dtype=np.float32)
    mask = np.asarray(attention_mask, dtype=np.float32)
    Wq = np.asarray(Wq, dtype=np.float32)
    Wk = np.asarray(Wk, dtype=np.float32)
    Wv = np.asarray(Wv, dtype=np.float32)
    Wo = np.asarray(Wo, dtype=np.float32)
    out, _ = run(hs, mask, Wq, Wk, Wv, Wo)
    return out


# revision 34
# speedup vs baseline: 1.0017x; 1.0017x over previous
"""Trainium2 Bass kernel for nn_DualWeightAttention (B=2, S=2048, H=2048, 16 heads).

Sharding: tensor-parallel over heads — 2 heads per core on 8 cores.
Each core computes q/k/v projections for its 2 heads, attention for those
heads (both batches), and a partial output projection against its 256-row
slice of Wo.T. The 8 partial [4096, 2048] fp16 outputs are summed on the host
in f32.

v2 schedule: the projection phase and the attention phase are MERGED so the
PE never starves while ScalarE does the exp stream:
  - Phase A projects batch 0 only (~82us of PE work, baseline structure).
  - The 16 attention periods then interleave, per period:
      QK(i+1) kt-pair matmuls -> exp -> mask-mult   (ACT/DVE paced)
      PV(i) accumulation                            (PE)
      batch-1 projection groups (periods 0-7, one 16-matmul group per
      2 kt-pairs, single shared PSUM bank)          (PE filler)
      output-projection tiles drawn from a BACKLOG queue paced so late
      periods (no projection filler left) stay PE-bound (PE filler)
  - Softmax denominator: DVE tree folds the attn slab to [128(k), 512(q)],
    then ONE GpSimd partition_all_reduce produces the broadcast row-sums
    [128, 512] directly (replaces the baseline's ones-matmul + reciprocal +
    partition_broadcast chain and frees a PSUM bank). Launched at j==3 of
    the unit's own period so its ~3us latency hides before uT_mult needs it.
  - Outputs are written as fp16 partials (halves output DMA; host sums in
    f32; adds ~3e-4 rel error, budget is 2e-2).

PSUM budget (8 banks): QK pairs 2x[P,2,QC]=4, PV accumulators 2, outproj 1,
b1-projection shared bank 1.
"""

import numpy as np

import concourse.mybir as mybir
import concourse.tile as tile
from concourse import bacc
from concourse import bass_isa
from concourse.bass_utils import run_bass_kernel_spmd

P = 128
B = 2
S = 2048
H = 2048
NH = 16
HD = 128
NCORES = 8
HPC = NH // NCORES  # heads per core
DC = HPC * HD       # d-columns per core
QC = 512            # q-chunk (matmul moving free dim)
HT = H // P         # contraction tiles for projections
SCALE = 1.0 / float(np.sqrt(HD))

F32 = mybir.dt.float32
BF16 = mybir.dt.float16  # fp16 over bf16: same PE/DVE rates, finer mantissa

PROJ_DT = BF16  # hsT + wq/wk/wv
QK_DT = BF16    # qT/kT operands
OUT_DT = BF16   # uT + woT
MASK_DT = BF16
EXP = mybir.ActivationFunctionType.Exp
ADD = mybir.AluOpType.add
MULT = mybir.AluOpType.mult


def build_attention_nc(s=S):
    bs = B * s
    kt_n = s // P     # k tiles per batch
    kp_n = kt_n // 2  # kt pairs
    nq = s // QC      # q chunks per batch
    st_n = s // P     # s tiles per batch (out projection)
    vt_n = bs // P    # v tiles (both batches)
    KH = kt_n // 2    # kt per mask half
    NQT = 4
    KOQ = HT // NQT   # hsT streamed as 4 quarter-K tiles per s-chunk
    NOUT = (QC // P) * (H // QC)  # outproj tiles per chunk (16)
    NCH = B * nq      # total chunks (8)

    nc = bacc.Bacc("TRN2", target_bir_lowering=False, debug=False, num_devices=NCORES)
    hsT = nc.dram_tensor("hsT", [H, bs], PROJ_DT, kind="ExternalInput")
    maskT = nc.dram_tensor("maskT", [B, s, s], MASK_DT, kind="ExternalInput")
    wqT = nc.dram_tensor("wqT", [H, DC], PROJ_DT, kind="ExternalInput")
    wkT = nc.dram_tensor("wkT", [H, DC], PROJ_DT, kind="ExternalInput")
    wvT = nc.dram_tensor("wvT", [H, DC], PROJ_DT, kind="ExternalInput")
    woT = nc.dram_tensor("woT", [DC, H], OUT_DT, kind="ExternalInput")
    out = nc.dram_tensor("out", [bs, H], OUT_DT, kind="ExternalOutput")

    hsT_r = hsT.ap().rearrange("(o p) t -> p o t", p=P)
    wq_r = wqT.ap().rearrange("(o p) d -> p o d", p=P)
    wk_r = wkT.ap().rearrange("(o p) d -> p o d", p=P)
    wv_r = wvT.ap().rearrange("(o p) d -> p o d", p=P)
    wo_r = woT.ap().rearrange("(h p) j -> p h j", p=P)
    out_r = out.ap().rearrange("(t p) j -> p t j", p=P)

    with tile.TileContext(nc) as tc:
        with (
            tc.tile_pool(name="persist", bufs=1) as persist,
        ):
            qT = persist.tile([P, HPC, bs], QK_DT)
            kT = persist.tile([P, HPC, bs], QK_DT)
            vsb = persist.tile([P, vt_n, DC], BF16)
            wo_sb = persist.tile([P, HPC, H], OUT_DT)
            wq_sb = persist.tile([P, HT, DC], PROJ_DT, name="wq_sb")
            wk_sb = persist.tile([P, HT, DC], PROJ_DT, name="wk_sb")
            wv_sb = persist.tile([P, HT, DC], PROJ_DT, name="wv_sb")

            # evacuation helper: alternate DVE/ACT so neither paces the PE,
            # with an optional forced engine for load balancing
            _ev = [0]

            def evac(dst, src, eng=None):
                if eng is None:
                    eng = "s" if _ev[0] % 2 == 0 else "v"
                    _ev[0] += 1
                if eng == "s":
                    nc.scalar.copy(dst, src)
                else:
                    nc.vector.tensor_copy(dst, src)

            # hsT quarter tiles stream through a ring shared by both phases
            hpool_cm = tc.tile_pool(name="hpool", bufs=6)
            hpool = hpool_cm.__enter__()
            quarters = {}  # sc -> [4 quarter tiles]

            def hst_issue(sc, eng=None):
                eng = eng or nc.sync
                ssl = slice(sc * QC, (sc + 1) * QC)
                qs = []
                for qf in range(NQT):
                    hst = hpool.tile([P, KOQ, QC], PROJ_DT, tag="hst", name="hst")
                    eng.dma_start(
                        hst[:], hsT_r[:, qf * KOQ : (qf + 1) * KOQ, ssl]
                    )
                    qs.append(hst)
                quarters[sc] = qs

            def hq(sc, ko):
                return quarters[sc][ko // KOQ][:, ko % KOQ]

            # mask halves: ring of 3, issued one per period start
            mpool_cm = tc.tile_pool(name="mpool", bufs=3)
            mpool = mpool_cm.__enter__()
            mhalves = {}  # (chunk, mh) -> tile

            def mask_issue(ch, mh):
                if (ch, mh) in mhalves:
                    return
                b, qq = divmod(ch, nq)
                ms = mpool.tile([P, KH, QC], MASK_DT, tag="mslab", name="ms")
                nc.sync.dma_start(
                    ms[:],
                    maskT.ap()[b].rearrange("(kt p) q -> p kt q", p=P)[
                        :, mh * KH : (mh + 1) * KH,
                        qq * QC : (qq + 1) * QC,
                    ],
                )
                mhalves[(ch, mh)] = ms

            # ---------------- Phase A: batch-0 projections ----------------
            with (
                tc.tile_pool(name="ppsum", bufs=2, space="PSUM") as ppsum,
                tc.tile_pool(name="vpsum", bufs=4, space="PSUM") as vpsum,
            ):
                # DMA order: first q-projection group needs wq quarter 0 and
                # the first hsT quarter — issue those first.
                # issue the critical first tiles on TWO DGE queues in
                # parallel: wq quarter 0 on Sync, hsT chunk 0 on ScalarE
                # (idle at startup); the rest follows on Sync
                nc.sync.dma_start(wq_sb[:, 0:4], wq_r[:, 0:4])
                hst_issue(0, nc.scalar)
                for _wf in range(1, 4):
                    _wsl = slice(_wf * (HT // 4), (_wf + 1) * (HT // 4))
                    nc.sync.dma_start(wq_sb[:, _wsl], wq_r[:, _wsl])
                nc.sync.dma_start(wk_sb[:], wk_r)
                nc.sync.dma_start(wv_sb[:], wv_r)
                nc.sync.dma_start(wo_sb[:], wo_r)

                for sc in range(nq):  # batch 0 chunks only
                    if sc > 0:
                        hst_issue(sc)
                    ssl = slice(sc * QC, (sc + 1) * QC)
                    if sc == 2:
                        mask_issue(0, 0)
                        mask_issue(0, 1)
                    # q(h0), q(h1), k(h0), k(h1) — q first so the wk DMA has
                    # more slack at startup
                    for wsb, dstT in ((wq_sb, qT), (wk_sb, kT)):
                        for h in range(HPC):
                            ps = ppsum.tile([P, QC], F32, tag="psqk")
                            for ko in range(HT):
                                nc.tensor.matmul(
                                    ps[:],
                                    wsb[:, ko, h * P : (h + 1) * P],
                                    hq(sc, ko),
                                    start=(ko == 0),
                                    stop=(ko == HT - 1),
                                )
                            evac(dstT[:, h, ssl], ps[:])
                    # v: ko-outer over 4 concurrent PSUM groups
                    psvs = []
                    for st in range(QC // P):
                        psv = vpsum.tile([P, DC], F32, tag="psv")
                        psvs.append(psv)
                    for ko in range(HT):
                        for st in range(QC // P):
                            nc.tensor.matmul(
                                psvs[st][:],
                                hq(sc, ko)[:, st * P : (st + 1) * P],
                                wv_sb[:, ko, :],
                                start=(ko == 0),
                                stop=(ko == HT - 1),
                            )
                    for st in range(QC // P):
                        evac(vsb[:, sc * (QC // P) + st, :], psvs[st][:])
                # prefetch first b1 chunk's hsT for the phase-2 filler
                hst_issue(nq)

            # ---------------- Phase 2: merged attention + b1 projections ----------------
            with (
                tc.tile_pool(name="apool", bufs=2) as apool,
                tc.tile_pool(name="tpool", bufs=2) as tpool,
                tc.tile_pool(name="upool", bufs=2) as upool,
                tc.tile_pool(name="rpool", bufs=1) as rpool,
                tc.tile_pool(name="opool", bufs=6) as opool,
                tc.tile_pool(name="spsum", bufs=2, space="PSUM") as spsum,
                tc.tile_pool(name="upsum", bufs=2, space="PSUM") as upsum,
            ):
                # single-bank pools for periods 0-7 (outproj tiles + b1
                # projections), manually scoped: both release after period 7
                # and their two banks recycle as ONE two-bank outproj pool so
                # late-period tiles are drawn in adjacent-jc PAIRS with a
                # single [128,1024] evac + single DMA per pair
                opsum_cm = tc.tile_pool(name="opsum", bufs=1, space="PSUM")
                opsum = opsum_cm.__enter__()
                pproj_cm = tc.tile_pool(name="pproj", bufs=1, space="PSUM")
                pproj = pproj_cm.__enter__()
                odsum_holder = [None, None]  # (cm, pool)
                units = [
                    (b, qq, h)
                    for b in range(B)
                    for qq in range(nq)
                    for h in range(HPC)
                ]
                nu = len(units)
                aslabs = {}
                psus = {}
                s8s = {}
                rsums = {}
                rbcs = {}
                uTs = {}

                def qk_pair(i, j):
                    # two scoresT k-tile matmuls into one 2-bank psum tile;
                    # exp(s+m) = exp(s)*exp(m): ScalarE exp evacuates the
                    # [128,1024] pair in one ACT op; the host-precomputed
                    # exp(mask) factor is one fp16 [128,1024] DVE multiply
                    b, qq, h = units[i]
                    ch = i // HPC
                    if j == 0:
                        asl = apool.tile([P, kt_n, QC], BF16, tag="aslab", name="asl")
                        aslabs[i] = asl
                    asl = aslabs[i]
                    pss = spsum.tile([P, 2, QC], F32, tag="pss")
                    for u in range(2):
                        kt = 2 * j + u
                        nc.tensor.matmul(
                            pss[:, u],
                            kT[:, h, b * s + kt * P : b * s + (kt + 1) * P],
                            qT[:, h, b * s + qq * QC : b * s + (qq + 1) * QC],
                            start=True,
                            stop=True,
                        )
                    nc.scalar.activation(asl[:, 2 * j : 2 * j + 2], pss[:], EXP)
                    # mask multiply batched over TWO pairs ([128, 2048] per
                    # DVE op, 4 ops/unit instead of 8 — amortizes op
                    # overhead; 4-slab groups align exactly with mask halves)
                    if j % 2 == 1:
                        ms = mhalves[(ch, (2 * j) // KH)]
                        mo = (2 * j - 2) % KH
                        nc.vector.tensor_tensor(
                            asl[:, 2 * j - 2 : 2 * j + 2],
                            asl[:, 2 * j - 2 : 2 * j + 2],
                            ms[:, mo : mo + 4],
                            MULT,
                        )

                def pv_part(i, kt):
                    b, qq, h = units[i]
                    asl = aslabs[i]
                    if kt == 0:
                        psu = upsum.tile([P, QC], F32, tag="psu")
                        psus[i] = psu
                    nc.tensor.matmul(
                        psus[i][:],
                        vsb[:, b * kt_n + kt, h * P : (h + 1) * P],
                        asl[:, kt],
                        start=(kt == 0),
                        stop=(kt == kt_n - 1),
                    )

                def den_tree_level(i, lvl):
                    # kt-fold of the attn slab into a scratch tile (asl is
                    # fully written at period start); fp16 2x DVE adds
                    asl = aslabs[i]
                    if lvl == 0:
                        s8 = tpool.tile([P, KH, QC], BF16, tag="s8", name="s8")
                        s8s[i] = s8
                        nc.vector.tensor_tensor(
                            s8[:], asl[:, 0:KH], asl[:, KH : 2 * KH], ADD
                        )
                    else:
                        s8 = s8s[i]
                        w = KH >> lvl
                        nc.vector.tensor_tensor(
                            s8[:, 0:w], s8[:, 0:w], s8[:, w : 2 * w], ADD
                        )

                def den_all_reduce(i):
                    # ONE GpSimd partition_all_reduce turns the folded
                    # [128(k), 512(q)] tile into broadcast row sums [128, 512]
                    rsum = rpool.tile([P, QC], F32, tag="rsum", name="rsum")
                    nc.gpsimd.partition_all_reduce(
                        rsum[:], s8s.pop(i)[:, 0], 128, bass_isa.ReduceOp.add
                    )
                    rsums[i] = rsum

                def den_recip(i):
                    rbc = rpool.tile([P, QC], F32, tag="rbc", name="rbc")
                    nc.vector.reciprocal_approx_fast(out=rbc[:], in_=rsums.pop(i)[:])
                    rbcs[i] = rbc

                def uT_mult(i):
                    # normalize + evacuate the PV accumulator in one DVE op
                    b, qq, h = units[i]
                    if b not in uTs:
                        uT_new = upool.tile([P, HPC, s], OUT_DT, tag="uT", name="uT")
                        uTs[b] = uT_new
                    nc.vector.tensor_tensor(
                        uTs[b][:, h, qq * QC : (qq + 1) * QC],
                        psus.pop(i)[:],
                        rbcs.pop(i)[:],
                        MULT,
                    )

                _ot_n = [0]

                def outproj_tile(b, qq, t, pso=None, eng=None):
                    # one output-projection tile: 2 accumulating matmuls
                    # (h=0,1) + evac + DMA out; late periods alternate two
                    # independent banks so the PE never waits an evac
                    st_l, jc = divmod(t, H // QC)
                    st = qq * (QC // P) + st_l
                    uT_b = uTs[b]
                    if pso is None:
                        o2 = odsum_holder[1]
                        if o2 is not None and _ot_n[0] % 2 == 1:
                            pso = o2.tile([P, QC], F32, tag="pso2", name="pso2")
                        else:
                            pso = opsum.tile([P, QC], F32, tag="pso", name="pso")
                        _ot_n[0] += 1
                    for h in range(HPC):
                        nc.tensor.matmul(
                            pso[:],
                            uT_b[:, h, st * P : (st + 1) * P],
                            wo_sb[:, h, jc * QC : (jc + 1) * QC],
                            start=(h == 0),
                            stop=(h == HPC - 1),
                        )
                    ot = opool.tile([P, QC], OUT_DT, tag="ot", name="ot", bufs=3)
                    evac(ot[:], pso[:], eng=eng)
                    nc.sync.dma_start(
                        out_r[:, b * st_n + st, jc * QC : (jc + 1) * QC], ot[:]
                    )

                def outproj_pair(b, qq, t0, pso2, eng=None):
                    # two adjacent-jc tiles into a 2-bank psum tile: 4 MMs,
                    # ONE [128,1024] evac, ONE contiguous DMA (drain only)
                    st_l, jc = divmod(t0, H // QC)
                    st = qq * (QC // P) + st_l
                    uT_b = uTs[b]
                    for u in range(2):
                        for h in range(HPC):
                            nc.tensor.matmul(
                                pso2[:, u],
                                uT_b[:, h, st * P : (st + 1) * P],
                                wo_sb[:, h, (jc + u) * QC : (jc + u + 1) * QC],
                                start=(h == 0),
                                stop=(h == HPC - 1),
                            )
                    ot = opool.tile([P, 2, QC], OUT_DT, tag="ot2", name="ot2", bufs=2)
                    evac(ot[:], pso2[:], eng=eng)
                    nc.sync.dma_start(
                        out_r[:, b * st_n + st, jc * QC : (jc + 2) * QC], ot[:]
                    )

                # ---- b1 projection filler groups (shared single PSUM bank)
                def proj_group_qk(wsb, dstT, h, sc):
                    ps = pproj.tile([P, QC], F32, tag="pj", name="pj")
                    ssl = slice(sc * QC, (sc + 1) * QC)
                    for ko in range(HT):
                        nc.tensor.matmul(
                            ps[:],
                            wsb[:, ko, h * P : (h + 1) * P],
                            hq(sc, ko),
                            start=(ko == 0),
                            stop=(ko == HT - 1),
                        )
                    evac(dstT[:, h, ssl], ps[:])

                def proj_group_v(st, sc):
                    psv = pproj.tile([P, DC], F32, tag="pj", name="pj")
                    for ko in range(HT):
                        nc.tensor.matmul(
                            psv[:],
                            hq(sc, ko)[:, st * P : (st + 1) * P],
                            wv_sb[:, ko, :],
                            start=(ko == 0),
                            stop=(ko == HT - 1),
                        )
                    evac(vsb[:, sc * (QC // P) + st, :], psv[:])

                proj_jobs = []
                for sc in range(nq, 2 * nq):  # b1 global chunks 4..7
                    proj_jobs.append(("qk", wq_sb, qT, 0, sc))
                    proj_jobs.append(("qk", wq_sb, qT, 1, sc))
                    proj_jobs.append(("qk", wk_sb, kT, 0, sc))
                    proj_jobs.append(("qk", wk_sb, kT, 1, sc))
                    proj_jobs.append(("v", None, None, 0, sc))
                    proj_jobs.append(("v", None, None, 1, sc))
                    proj_jobs.append(("v", None, None, 2, sc))
                    proj_jobs.append(("v", None, None, 3, sc))
                _pj = [0]

                def emit_proj_job():
                    if _pj[0] >= len(proj_jobs):
                        return
                    kind, wsb, dstT, x, sc = proj_jobs[_pj[0]]
                    _pj[0] += 1
                    if kind == "qk":
                        proj_group_qk(wsb, dstT, x, sc)
                    else:
                        proj_group_v(x, sc)
                    # prefetch the next b1 chunk's hsT once this chunk's last
                    # group is emitted
                    if _pj[0] % 8 == 0 and sc + 1 < 2 * nq:
                        hst_issue(sc + 1)

                # outproj backlog pacing: uniform 8/period (availability
                # capped: chunk c's 16 tiles appear at period 2c+2). Early
                # periods draw singles at even j (projection groups own odd
                # j); late periods draw PAIRS at odd j.
                DRAW = [0, 0] + [8] * 14
                JD_EARLY = [0, 0, 2, 0, 2, 0, 2, 2]
                JD_LATE = [1, 1, 1, 1, 1, 1, 1, 1]
                backlog = []
                # outproj evac engine pattern: biased to ScalarE (its exp
                # stream leaves it more slack than DVE's mask+tree+uT+recip)
                EVPAT = ("s", "s", "v")

                # ---- pipeline prologue: unit 0's QK pairs
                for j in range(kp_n):
                    qk_pair(0, j)

                for i in range(nu):
                    b, qq, h = units[i]
                    if i == 8:
                        # projections done: recycle the pproj bank as a
                        # second outproj bank (alternating with opsum)
                        pproj_cm.__exit__(None, None, None)
                        cm = tc.tile_pool(name="odsum", bufs=1, space="PSUM")
                        odsum_holder[0] = cm
                        odsum_holder[1] = cm.__enter__()
                    # mask prefetch: chunk i//2+1, half i%2
                    ch_next = i // 2 + 1
                    if ch_next < NCH:
                        mask_issue(ch_next, i % 2)
                    # denominator chain of i-1 + uT evac (rbc long ready)
                    if i >= 1:
                        den_recip(i - 1)
                        uT_mult(i - 1)
                        if units[i - 1][2] == HPC - 1:
                            pc = (i - 1) // 2
                            pb, pqq = pc // nq, pc % nq
                            backlog.extend((pb, pqq, t) for t in range(NOUT))
                    jdist = JD_EARLY if i < 8 else JD_LATE
                    ndraw = 0
                    for j in range(kp_n):
                        if i + 1 < nu:
                            qk_pair(i + 1, j)
                        pv_part(i, 2 * j)
                        pv_part(i, 2 * j + 1)
                        if j <= 3:
                            den_tree_level(i, j)
                        if j == 3:
                            den_all_reduce(i)
                        for _ in range(jdist[j]):
                            if backlog and ndraw < DRAW[i]:
                                pb, pqq, t = backlog.pop(0)
                                outproj_tile(
                                    pb, pqq, t, eng=EVPAT[ndraw % len(EVPAT)]
                                )
                                ndraw += 1
                        if i < 8 and j % 2 == 1:
                            emit_proj_job()
                # drain: last unit's den chain + final chunk's outproj with
                # deep psum pipelining (borrow the freed QK pair banks)
                den_recip(nu - 1)
                uT_mult(nu - 1)
                rest = list(backlog) + [(B - 1, nq - 1, t) for t in range(NOUT)]
                for n in range(0, len(rest), 2):
                    ob, oqq, t0 = rest[n]
                    pso2 = spsum.tile([P, 2, QC], F32, tag="pss", name="big")
                    outproj_pair(ob, oqq, t0, pso2=pso2)
                odsum_holder[0].__exit__(None, None, None)
                opsum_cm.__exit__(None, None, None)

            mpool_cm.__exit__(None, None, None)
            hpool_cm.__exit__(None, None, None)
    nc.compile()
    return nc


def make_in_maps(hs, mask, Wq, Wk, Wv, Wo):
    """Host-side prep: transpose/shard the full inputs into per-core maps."""
    bs = hs.shape[0] * hs.shape[1]
    proj_np = np.float16
    out_np = np.float16
    hsT = np.ascontiguousarray(hs.reshape(bs, H).T).astype(proj_np)
    maskT = np.exp(
        np.ascontiguousarray(mask[:, 0].transpose(0, 2, 1))
    ).astype(np.float16)
    in_maps = []
    for c in range(NCORES):
        sl = slice(c * DC, (c + 1) * DC)
        in_maps.append(
            {
                "hsT": hsT,
                "maskT": maskT,
                "wqT": np.ascontiguousarray((Wq[sl] * SCALE).T).astype(proj_np),
                "wkT": np.ascontiguousarray(Wk[sl].T).astype(proj_np),
                "wvT": np.ascontiguousarray(Wv[sl].T).astype(proj_np),
                "woT": np.ascontiguousarray(Wo[:, sl].T).astype(out_np),
            }
        )
    return in_maps


_NC_CACHE = {}


def get_nc(s=S):
    if s not in _NC_CACHE:
        _NC_CACHE[s] = build_attention_nc(s)
    return _NC_CACHE[s]


def run(hs, mask, Wq, Wk, Wv, Wo, trace=False, trace_kwargs=None):
    s = hs.shape[1]
    nc = get_nc(s)
    in_maps = make_in_maps(hs, mask, Wq, Wk, Wv, Wo)
    res = run_bass_kernel_spmd(
        nc,
        in_maps,
        core_ids=list(range(NCORES)),
        trace=trace,
        **(trace_kwargs or {}),
    )
    parts = np.stack([r["out"] for r in res.results])
    full = parts.astype(np.float32).sum(axis=0)
    return full.reshape(hs.shape[0], s, H), res


def kernel(hidden_states, attention_mask, Wq, Wk, Wv, Wo):
    hs = np.asarray(hidden_states, dtype=np.float32)
    mask = np.asarray(attention_mask, dtype=np.float32)
    Wq = np.asarray(Wq, dtype=np.float32)
    Wk = np.asarray(Wk, dtype=np.float32)
    Wv = np.asarray(Wv, dtype=np.float32)
    Wo = np.asarray(Wo, dtype=np.float32)
    out, _ = run(hs, mask, Wq, Wk, Wv, Wo)
    return out


# revision 36
# speedup vs baseline: 1.0249x; 1.0231x over previous
"""Trainium2 Bass kernel for nn_DualWeightAttention (B=2, S=2048, H=2048, 16 heads).

Sharding: tensor-parallel over heads — 2 heads per core on 8 cores.
Each core computes q/k/v projections for its 2 heads, attention for those
heads (both batches), and a partial output projection against its 256-row
slice of Wo.T. The 8 partial [4096, 2048] fp16 outputs are summed on the host
in f32.

v2 schedule: the projection phase and the attention phase are MERGED so the
PE never starves while ScalarE does the exp stream:
  - Phase A projects batch 0 only (~82us of PE work, baseline structure).
  - The 16 attention periods then interleave, per period:
      QK(i+1) kt-pair matmuls -> exp -> mask-mult   (ACT/DVE paced)
      PV(i) accumulation                            (PE)
      batch-1 projection groups (periods 0-7, one 16-matmul group per
      2 kt-pairs, single shared PSUM bank)          (PE filler)
      output-projection tiles drawn from a BACKLOG queue paced so late
      periods (no projection filler left) stay PE-bound (PE filler)
  - Softmax denominator: DVE tree folds the attn slab to [128(k), 512(q)],
    then ONE GpSimd partition_all_reduce produces the broadcast row-sums
    [128, 512] directly (replaces the baseline's ones-matmul + reciprocal +
    partition_broadcast chain and frees a PSUM bank). Launched at j==3 of
    the unit's own period so its ~3us latency hides before uT_mult needs it.
  - Outputs are written as fp16 partials (halves output DMA; host sums in
    f32; adds ~3e-4 rel error, budget is 2e-2).

PSUM budget (8 banks): QK pairs 2x[P,2,QC]=4, PV accumulators 2, outproj 1,
b1-projection shared bank 1.
"""

import numpy as np

import concourse.mybir as mybir
import concourse.tile as tile
from concourse import bacc
from concourse import bass_isa
from concourse.bass_utils import run_bass_kernel_spmd

P = 128
B = 2
S = 2048
H = 2048
NH = 16
HD = 128
NCORES = 8
HPC = NH // NCORES  # heads per core
DC = HPC * HD       # d-columns per core
QC = 512            # q-chunk (matmul moving free dim)
HT = H // P         # contraction tiles for projections
SCALE = 1.0 / float(np.sqrt(HD))

F32 = mybir.dt.float32
BF16 = mybir.dt.float16  # fp16 over bf16: same PE/DVE rates, finer mantissa

PROJ_DT = BF16  # hsT + wq/wk/wv
QK_DT = BF16    # qT/kT operands
OUT_DT = BF16   # uT + woT
MASK_DT = BF16
EXP = mybir.ActivationFunctionType.Exp
ADD = mybir.AluOpType.add
MULT = mybir.AluOpType.mult


def build_attention_nc(s=S):
    bs = B * s
    kt_n = s // P     # k tiles per batch
    kp_n = kt_n // 2  # kt pairs
    nq = s // QC      # q chunks per batch
    st_n = s // P     # s tiles per batch (out projection)
    vt_n = bs // P    # v tiles (both batches)
    KH = kt_n // 2    # kt per mask half
    NQT = 4
    KOQ = HT // NQT   # hsT streamed as 4 quarter-K tiles per s-chunk
    NOUT = (QC // P) * (H // QC)  # outproj tiles per chunk (16)
    NCH = B * nq      # total chunks (8)

    nc = bacc.Bacc("TRN2", target_bir_lowering=False, debug=False, num_devices=NCORES)
    hsT = nc.dram_tensor("hsT", [H, bs], PROJ_DT, kind="ExternalInput")
    maskT = nc.dram_tensor("maskT", [B, s, s], MASK_DT, kind="ExternalInput")
    wqT = nc.dram_tensor("wqT", [H, DC], PROJ_DT, kind="ExternalInput")
    wkT = nc.dram_tensor("wkT", [H, DC], PROJ_DT, kind="ExternalInput")
    wvT = nc.dram_tensor("wvT", [H, DC], PROJ_DT, kind="ExternalInput")
    woT = nc.dram_tensor("woT", [DC, H], OUT_DT, kind="ExternalInput")
    out = nc.dram_tensor("out", [bs, H], OUT_DT, kind="ExternalOutput")

    hsT_r = hsT.ap().rearrange("(o p) t -> p o t", p=P)
    wq_r = wqT.ap().rearrange("(o p) d -> p o d", p=P)
    wk_r = wkT.ap().rearrange("(o p) d -> p o d", p=P)
    wv_r = wvT.ap().rearrange("(o p) d -> p o d", p=P)
    wo_r = woT.ap().rearrange("(h p) j -> p h j", p=P)
    out_r = out.ap().rearrange("(t p) j -> p t j", p=P)

    with tile.TileContext(nc) as tc:
        with (
            tc.tile_pool(name="persist", bufs=1) as persist,
        ):
            qT = persist.tile([P, HPC, bs], QK_DT)
            kT = persist.tile([P, HPC, bs], QK_DT)
            vsb = persist.tile([P, vt_n, DC], BF16)
            wo_sb = persist.tile([P, HPC, H], OUT_DT)
            wq_sb = persist.tile([P, HT, DC], PROJ_DT, name="wq_sb")
            wk_sb = persist.tile([P, HT, DC], PROJ_DT, name="wk_sb")
            wv_sb = persist.tile([P, HT, DC], PROJ_DT, name="wv_sb")

            # evacuation helper: alternate DVE/ACT so neither paces the PE,
            # with an optional forced engine for load balancing
            _ev = [0]

            def evac(dst, src, eng=None):
                if eng is None:
                    eng = "s" if _ev[0] % 2 == 0 else "v"
                    _ev[0] += 1
                if eng == "s":
                    nc.scalar.copy(dst, src)
                else:
                    nc.vector.tensor_copy(dst, src)

            # hsT quarter tiles stream through a ring shared by both phases
            hpool_cm = tc.tile_pool(name="hpool", bufs=7)
            hpool = hpool_cm.__enter__()
            quarters = {}  # sc -> [4 quarter tiles]

            def hst_issue(sc, eng=None):
                eng = eng or nc.sync
                ssl = slice(sc * QC, (sc + 1) * QC)
                qs = []
                for qf in range(NQT):
                    hst = hpool.tile([P, KOQ, QC], PROJ_DT, tag="hst", name="hst")
                    eng.dma_start(
                        hst[:], hsT_r[:, qf * KOQ : (qf + 1) * KOQ, ssl]
                    )
                    qs.append(hst)
                quarters[sc] = qs

            def hq(sc, ko):
                return quarters[sc][ko // KOQ][:, ko % KOQ]

            # mask halves: ring of 3, issued one per period start
            mpool_cm = tc.tile_pool(name="mpool", bufs=3)
            mpool = mpool_cm.__enter__()
            mhalves = {}  # (chunk, mh) -> tile

            def mask_issue(ch, mh):
                if (ch, mh) in mhalves:
                    return
                b, qq = divmod(ch, nq)
                ms = mpool.tile([P, KH, QC], MASK_DT, tag="mslab", name="ms")
                nc.sync.dma_start(
                    ms[:],
                    maskT.ap()[b].rearrange("(kt p) q -> p kt q", p=P)[
                        :, mh * KH : (mh + 1) * KH,
                        qq * QC : (qq + 1) * QC,
                    ],
                )
                mhalves[(ch, mh)] = ms

            # ---------------- Phase A: batch-0 projections ----------------
            with (
                tc.tile_pool(name="ppsum", bufs=2, space="PSUM") as ppsum,
                tc.tile_pool(name="vpsum", bufs=4, space="PSUM") as vpsum,
            ):
                # DMA order: first q-projection group needs wq quarter 0 and
                # the first hsT quarter — issue those first.
                # issue the critical first tiles on TWO DGE queues in
                # parallel: wq quarter 0 on Sync, hsT chunk 0 on ScalarE
                # (idle at startup); the rest follows on Sync
                nc.sync.dma_start(wq_sb[:, 0:4], wq_r[:, 0:4])
                hst_issue(0, nc.scalar)
                for _wf in range(1, 4):
                    _wsl = slice(_wf * (HT // 4), (_wf + 1) * (HT // 4))
                    nc.sync.dma_start(wq_sb[:, _wsl], wq_r[:, _wsl])
                nc.sync.dma_start(wk_sb[:], wk_r)
                nc.sync.dma_start(wv_sb[:], wv_r)
                nc.sync.dma_start(wo_sb[:], wo_r)

                for sc in range(nq):  # batch 0 chunks only
                    if sc > 0:
                        hst_issue(sc)
                    ssl = slice(sc * QC, (sc + 1) * QC)
                    if sc == 2:
                        mask_issue(0, 0)
                        mask_issue(0, 1)
                    # q(h0), q(h1), k(h0), k(h1) — q first so the wk DMA has
                    # more slack at startup
                    for wsb, dstT in ((wq_sb, qT), (wk_sb, kT)):
                        for h in range(HPC):
                            ps = ppsum.tile([P, QC], F32, tag="psqk")
                            for ko in range(HT):
                                nc.tensor.matmul(
                                    ps[:],
                                    wsb[:, ko, h * P : (h + 1) * P],
                                    hq(sc, ko),
                                    start=(ko == 0),
                                    stop=(ko == HT - 1),
                                )
                            evac(dstT[:, h, ssl], ps[:])
                    # v: ko-outer over 4 concurrent PSUM groups
                    psvs = []
                    for st in range(QC // P):
                        psv = vpsum.tile([P, DC], F32, tag="psv")
                        psvs.append(psv)
                    for ko in range(HT):
                        for st in range(QC // P):
                            nc.tensor.matmul(
                                psvs[st][:],
                                hq(sc, ko)[:, st * P : (st + 1) * P],
                                wv_sb[:, ko, :],
                                start=(ko == 0),
                                stop=(ko == HT - 1),
                            )
                    for st in range(QC // P):
                        evac(vsb[:, sc * (QC // P) + st, :], psvs[st][:])
                # prefetch first b1 chunk's hsT for the phase-2 filler
                hst_issue(nq)

            # ---------------- Phase 2: merged attention + b1 projections ----------------
            with (
                tc.tile_pool(name="apool", bufs=2) as apool,
                tc.tile_pool(name="tpool", bufs=1) as tpool,
                tc.tile_pool(name="upool", bufs=2) as upool,
                tc.tile_pool(name="rpool", bufs=1) as rpool,
                tc.tile_pool(name="opool", bufs=6) as opool,
                tc.tile_pool(name="spsum", bufs=2, space="PSUM") as spsum,
                tc.tile_pool(name="upsum", bufs=2, space="PSUM") as upsum,
            ):
                # single-bank pools for periods 0-7 (outproj tiles + b1
                # projections), manually scoped: both release after period 7
                # and their two banks recycle as ONE two-bank outproj pool so
                # late-period tiles are drawn in adjacent-jc PAIRS with a
                # single [128,1024] evac + single DMA per pair
                opsum_cm = tc.tile_pool(name="opsum", bufs=1, space="PSUM")
                opsum = opsum_cm.__enter__()
                pproj_cm = tc.tile_pool(name="pproj", bufs=1, space="PSUM")
                pproj = pproj_cm.__enter__()
                odsum_holder = [None, None]  # (cm, pool)
                units = [
                    (b, qq, h)
                    for b in range(B)
                    for qq in range(nq)
                    for h in range(HPC)
                ]
                nu = len(units)
                aslabs = {}
                psus = {}
                s8s = {}
                rsums = {}
                rbcs = {}
                uTs = {}

                def qk_pair(i, j):
                    # two scoresT k-tile matmuls into one 2-bank psum tile;
                    # exp(s+m) = exp(s)*exp(m): ScalarE exp evacuates the
                    # [128,1024] pair in one ACT op; the host-precomputed
                    # exp(mask) factor is one fp16 [128,1024] DVE multiply
                    b, qq, h = units[i]
                    ch = i // HPC
                    if j == 0:
                        asl = apool.tile([P, kt_n, QC], BF16, tag="aslab", name="asl")
                        aslabs[i] = asl
                    asl = aslabs[i]
                    pss = spsum.tile([P, 2, QC], F32, tag="pss")
                    for u in range(2):
                        kt = 2 * j + u
                        nc.tensor.matmul(
                            pss[:, u],
                            kT[:, h, b * s + kt * P : b * s + (kt + 1) * P],
                            qT[:, h, b * s + qq * QC : b * s + (qq + 1) * QC],
                            start=True,
                            stop=True,
                        )
                    nc.scalar.activation(asl[:, 2 * j : 2 * j + 2], pss[:], EXP)
                    # mask multiply batched over TWO pairs ([128, 2048] per
                    # DVE op, 4 ops/unit instead of 8 — amortizes op
                    # overhead; 4-slab groups align exactly with mask halves)
                    if j % 2 == 1:
                        ms = mhalves[(ch, (2 * j) // KH)]
                        mo = (2 * j - 2) % KH
                        nc.vector.tensor_tensor(
                            asl[:, 2 * j - 2 : 2 * j + 2],
                            asl[:, 2 * j - 2 : 2 * j + 2],
                            ms[:, mo : mo + 4],
                            MULT,
                        )

                def pv_part(i, kt):
                    b, qq, h = units[i]
                    asl = aslabs[i]
                    if kt == 0:
                        psu = upsum.tile([P, QC], F32, tag="psu")
                        psus[i] = psu
                    nc.tensor.matmul(
                        psus[i][:],
                        vsb[:, b * kt_n + kt, h * P : (h + 1) * P],
                        asl[:, kt],
                        start=(kt == 0),
                        stop=(kt == kt_n - 1),
                    )

                def den_tree_level(i, lvl):
                    # kt-fold of the attn slab into a scratch tile (asl is
                    # fully written at period start); fp16 2x DVE adds
                    asl = aslabs[i]
                    if lvl == 0:
                        s8 = tpool.tile([P, KH, QC], BF16, tag="s8", name="s8")
                        s8s[i] = s8
                        nc.vector.tensor_tensor(
                            s8[:], asl[:, 0:KH], asl[:, KH : 2 * KH], ADD
                        )
                    else:
                        s8 = s8s[i]
                        w = KH >> lvl
                        nc.vector.tensor_tensor(
                            s8[:, 0:w], s8[:, 0:w], s8[:, w : 2 * w], ADD
                        )

                def den_all_reduce(i):
                    # ONE GpSimd partition_all_reduce turns the folded
                    # [128(k), 512(q)] tile into broadcast row sums [128, 512]
                    rsum = rpool.tile([P, QC], F32, tag="rsum", name="rsum")
                    nc.gpsimd.partition_all_reduce(
                        rsum[:], s8s.pop(i)[:, 0], 128, bass_isa.ReduceOp.add
                    )
                    rsums[i] = rsum

                def den_recip(i):
                    rbc = rpool.tile([P, QC], F32, tag="rbc", name="rbc")
                    nc.vector.reciprocal_approx_fast(out=rbc[:], in_=rsums.pop(i)[:])
                    rbcs[i] = rbc

                def uT_mult(i):
                    # normalize + evacuate the PV accumulator in one DVE op
                    b, qq, h = units[i]
                    if b not in uTs:
                        uT_new = upool.tile([P, HPC, s], OUT_DT, tag="uT", name="uT")
                        uTs[b] = uT_new
                    nc.vector.tensor_tensor(
                        uTs[b][:, h, qq * QC : (qq + 1) * QC],
                        psus.pop(i)[:],
                        rbcs.pop(i)[:],
                        MULT,
                    )

                _ot_n = [0]

                def outproj_tile(b, qq, t, pso=None, eng=None):
                    # one output-projection tile: 2 accumulating matmuls
                    # (h=0,1) + evac + DMA out; late periods alternate two
                    # independent banks so the PE never waits an evac
                    st_l, jc = divmod(t, H // QC)
                    st = qq * (QC // P) + st_l
                    uT_b = uTs[b]
                    if pso is None:
                        o2 = odsum_holder[1]
                        if o2 is not None and _ot_n[0] % 2 == 1:
                            pso = o2.tile([P, QC], F32, tag="pso2", name="pso2")
                        else:
                            pso = opsum.tile([P, QC], F32, tag="pso", name="pso")
                        _ot_n[0] += 1
                    for h in range(HPC):
                        nc.tensor.matmul(
                            pso[:],
                            uT_b[:, h, st * P : (st + 1) * P],
                            wo_sb[:, h, jc * QC : (jc + 1) * QC],
                            start=(h == 0),
                            stop=(h == HPC - 1),
                        )
                    ot = opool.tile([P, QC], OUT_DT, tag="ot", name="ot", bufs=5)
                    evac(ot[:], pso[:], eng=eng)
                    nc.sync.dma_start(
                        out_r[:, b * st_n + st, jc * QC : (jc + 1) * QC], ot[:]
                    )

                def outproj_pair(b, qq, t0, pso2, eng=None):
                    # two adjacent-jc tiles into a 2-bank psum tile: 4 MMs,
                    # ONE [128,1024] evac, ONE contiguous DMA (drain only)
                    st_l, jc = divmod(t0, H // QC)
                    st = qq * (QC // P) + st_l
                    uT_b = uTs[b]
                    for u in range(2):
                        for h in range(HPC):
                            nc.tensor.matmul(
                                pso2[:, u],
                                uT_b[:, h, st * P : (st + 1) * P],
                                wo_sb[:, h, (jc + u) * QC : (jc + u + 1) * QC],
                                start=(h == 0),
                                stop=(h == HPC - 1),
                            )
                    ot = opool.tile([P, 2, QC], OUT_DT, tag="ot2", name="ot2", bufs=2)
                    evac(ot[:], pso2[:], eng=eng)
                    nc.sync.dma_start(
                        out_r[:, b * st_n + st, jc * QC : (jc + 2) * QC], ot[:]
                    )

                # ---- b1 projection filler groups (shared single PSUM bank)
                def proj_group_qk(wsb, dstT, h, sc):
                    ps = pproj.tile([P, QC], F32, tag="pj", name="pj")
                    ssl = slice(sc * QC, (sc + 1) * QC)
                    for ko in range(HT):
                        nc.tensor.matmul(
                            ps[:],
                            wsb[:, ko, h * P : (h + 1) * P],
                            hq(sc, ko),
                            start=(ko == 0),
                            stop=(ko == HT - 1),
                        )
                    evac(dstT[:, h, ssl], ps[:])

                def proj_group_v(st, sc):
                    psv = pproj.tile([P, DC], F32, tag="pj", name="pj")
                    for ko in range(HT):
                        nc.tensor.matmul(
                            psv[:],
                            hq(sc, ko)[:, st * P : (st + 1) * P],
                            wv_sb[:, ko, :],
                            start=(ko == 0),
                            stop=(ko == HT - 1),
                        )
                    evac(vsb[:, sc * (QC // P) + st, :], psv[:])

                proj_jobs = []
                for sc in range(nq, 2 * nq):  # b1 global chunks 4..7
                    proj_jobs.append(("qk", wq_sb, qT, 0, sc))
                    proj_jobs.append(("qk", wq_sb, qT, 1, sc))
                    proj_jobs.append(("qk", wk_sb, kT, 0, sc))
                    proj_jobs.append(("qk", wk_sb, kT, 1, sc))
                    proj_jobs.append(("v", None, None, 0, sc))
                    proj_jobs.append(("v", None, None, 1, sc))
                    proj_jobs.append(("v", None, None, 2, sc))
                    proj_jobs.append(("v", None, None, 3, sc))
                _pj = [0]

                def emit_proj_job():
                    if _pj[0] >= len(proj_jobs):
                        return
                    kind, wsb, dstT, x, sc = proj_jobs[_pj[0]]
                    _pj[0] += 1
                    if kind == "qk":
                        proj_group_qk(wsb, dstT, x, sc)
                    else:
                        proj_group_v(x, sc)
                    # prefetch the next b1 chunk's hsT once this chunk's last
                    # group is emitted
                    if _pj[0] % 8 == 0 and sc + 1 < 2 * nq:
                        hst_issue(sc + 1)

                # outproj backlog pacing: uniform 8/period (availability
                # capped: chunk c's 16 tiles appear at period 2c+2). Early
                # periods draw singles at even j (projection groups own odd
                # j); late periods draw PAIRS at odd j.
                DRAW = [0, 0] + [8] * 14
                JD_EARLY = [0, 0, 2, 0, 2, 0, 2, 2]
                JD_LATE = [1, 1, 1, 1, 1, 1, 1, 1]
                backlog = []
                # outproj evac engine pattern: biased to ScalarE (its exp
                # stream leaves it more slack than DVE's mask+tree+uT+recip)
                EVPAT = ("s", "s", "v")

                # ---- pipeline prologue: unit 0's QK pairs
                for j in range(kp_n):
                    qk_pair(0, j)

                for i in range(nu):
                    b, qq, h = units[i]
                    if i == 8:
                        # projections done: recycle the pproj bank as a
                        # second outproj bank (alternating with opsum)
                        pproj_cm.__exit__(None, None, None)
                        cm = tc.tile_pool(name="odsum", bufs=1, space="PSUM")
                        odsum_holder[0] = cm
                        odsum_holder[1] = cm.__enter__()
                    # mask prefetch: chunk i//2+1, half i%2
                    ch_next = i // 2 + 1
                    if ch_next < NCH:
                        mask_issue(ch_next, i % 2)
                    # denominator chain of i-1 + uT evac (rbc long ready)
                    if i >= 1:
                        den_recip(i - 1)
                        uT_mult(i - 1)
                        if units[i - 1][2] == HPC - 1:
                            pc = (i - 1) // 2
                            pb, pqq = pc // nq, pc % nq
                            backlog.extend((pb, pqq, t) for t in range(NOUT))
                    jdist = JD_EARLY if i < 8 else JD_LATE
                    ndraw = 0
                    for j in range(kp_n):
                        if i + 1 < nu:
                            qk_pair(i + 1, j)
                        pv_part(i, 2 * j)
                        pv_part(i, 2 * j + 1)
                        if j <= 3:
                            den_tree_level(i, j)
                        if j == 3:
                            den_all_reduce(i)
                        for _ in range(jdist[j]):
                            if backlog and ndraw < DRAW[i]:
                                pb, pqq, t = backlog.pop(0)
                                outproj_tile(
                                    pb, pqq, t, eng=EVPAT[ndraw % len(EVPAT)]
                                )
                                ndraw += 1
                        if i < 8 and j % 2 == 1:
                            emit_proj_job()
                # drain: last unit's den chain + final chunk's outproj with
                # deep psum pipelining (borrow the freed QK pair banks)
                den_recip(nu - 1)
                uT_mult(nu - 1)
                rest = list(backlog) + [(B - 1, nq - 1, t) for t in range(NOUT)]
                for n in range(0, len(rest), 2):
                    ob, oqq, t0 = rest[n]
                    pso2 = spsum.tile([P, 2, QC], F32, tag="pss", name="big")
                    outproj_pair(ob, oqq, t0, pso2=pso2)
                odsum_holder[0].__exit__(None, None, None)
                opsum_cm.__exit__(None, None, None)

            mpool_cm.__exit__(None, None, None)
            hpool_cm.__exit__(None, None, None)
    nc.compile()
    return nc


def make_in_maps(hs, mask, Wq, Wk, Wv, Wo):
    """Host-side prep: transpose/shard the full inputs into per-core maps."""
    bs = hs.shape[0] * hs.shape[1]
    proj_np = np.float16
    out_np = np.float16
    hsT = np.ascontiguousarray(hs.reshape(bs, H).T).astype(proj_np)
    maskT = np.exp(
        np.ascontiguousarray(mask[:, 0].transpose(0, 2, 1))
    ).astype(np.float16)
    in_maps = []
    for c in range(NCORES):
        sl = slice(c * DC, (c + 1) * DC)
        in_maps.append(
            {
                "hsT": hsT,
                "maskT": maskT,
                "wqT": np.ascontiguousarray((Wq[sl] * SCALE).T).astype(proj_np),
                "wkT": np.ascontiguousarray(Wk[sl].T).astype(proj_np),
                "wvT": np.ascontiguousarray(Wv[sl].T).astype(proj_np),
                "woT": np.ascontiguousarray(Wo[:, sl].T).astype(out_np),
            }
        )
    return in_maps


_NC_CACHE = {}


def get_nc(s=S):
    if s not in _NC_CACHE:
        _NC_CACHE[s] = build_attention_nc(s)
    return _NC_CACHE[s]


def run(hs, mask, Wq, Wk, Wv, Wo, trace=False, trace_kwargs=None):
    s = hs.shape[1]
    nc = get_nc(s)
    in_maps = make_in_maps(hs, mask, Wq, Wk, Wv, Wo)
    res = run_bass_kernel_spmd(
        nc,
        in_maps,
        core_ids=list(range(NCORES)),
        trace=trace,
        **(trace_kwargs or {}),
    )
    parts = np.stack([r["out"] for r in res.results])
    full = parts.astype(np.float32).sum(axis=0)
    return full.reshape(hs.shape[0], s, H), res


def kernel(hidden_states, attention_mask, Wq, Wk, Wv, Wo):
    hs = np.asarray(hidden_states, dtype=np.float32)
    mask = np.asarray(attention_mask, dtype=np.float32)
    Wq = np.asarray(Wq, dtype=np.float32)
    Wk = np.asarray(Wk, dtype=np.float32)
    Wv = np.asarray(Wv, dtype=np.float32)
    Wo = np.asarray(Wo, dtype=np.float32)
    out, _ = run(hs, mask, Wq, Wk, Wv, Wo)
    return out


# revision 39
# speedup vs baseline: 1.0508x; 1.0253x over previous
"""Trainium2 Bass kernel for nn_DualWeightAttention (B=2, S=2048, H=2048, 16 heads).

Sharding: tensor-parallel over heads — 2 heads per core on 8 cores.
Each core computes q/k/v projections for its 2 heads, attention for those
heads (both batches), and a partial output projection against its 256-row
slice of Wo.T. The 8 partial [4096, 2048] fp16 outputs are summed on the host
in f32.

v2 schedule: the projection phase and the attention phase are MERGED so the
PE never starves while ScalarE does the exp stream:
  - Phase A projects batch 0 only (~82us of PE work, baseline structure).
  - The 16 attention periods then interleave, per period:
      QK(i+1) kt-pair matmuls -> exp -> mask-mult   (ACT/DVE paced)
      PV(i) accumulation                            (PE)
      batch-1 projection groups (periods 0-7, one 16-matmul group per
      2 kt-pairs, single shared PSUM bank)          (PE filler)
      output-projection tiles drawn from a BACKLOG queue paced so late
      periods (no projection filler left) stay PE-bound (PE filler)
  - Softmax denominator: DVE tree folds the attn slab to [128(k), 512(q)],
    then ONE GpSimd partition_all_reduce produces the broadcast row-sums
    [128, 512] directly (replaces the baseline's ones-matmul + reciprocal +
    partition_broadcast chain and frees a PSUM bank). Launched at j==3 of
    the unit's own period so its ~3us latency hides before uT_mult needs it.
  - Outputs are written as fp16 partials (halves output DMA; host sums in
    f32; adds ~3e-4 rel error, budget is 2e-2).

PSUM budget (8 banks): QK pairs 2x[P,2,QC]=4, PV accumulators 2, outproj 1,
b1-projection shared bank 1.
"""

import numpy as np

import concourse.mybir as mybir
import concourse.tile as tile
from concourse import bacc
from concourse import bass_isa
from concourse.bass_utils import run_bass_kernel_spmd

P = 128
B = 2
S = 2048
H = 2048
NH = 16
HD = 128
NCORES = 8
HPC = NH // NCORES  # heads per core
DC = HPC * HD       # d-columns per core
QC = 512            # q-chunk (matmul moving free dim)
HT = H // P         # contraction tiles for projections
SCALE = 1.0 / float(np.sqrt(HD))

F32 = mybir.dt.float32
BF16 = mybir.dt.float16  # fp16 over bf16: same PE/DVE rates, finer mantissa

PROJ_DT = BF16  # hsT + wq/wk/wv
QK_DT = BF16    # qT/kT operands
OUT_DT = BF16   # uT + woT
MASK_DT = BF16
EXP = mybir.ActivationFunctionType.Exp
ADD = mybir.AluOpType.add
MULT = mybir.AluOpType.mult


def build_attention_nc(s=S):
    bs = B * s
    kt_n = s // P     # k tiles per batch
    kp_n = kt_n // 2  # kt pairs
    nq = s // QC      # q chunks per batch
    st_n = s // P     # s tiles per batch (out projection)
    vt_n = bs // P    # v tiles (both batches)
    KH = kt_n // 2    # kt per mask half
    NQT = 4
    KOQ = HT // NQT   # hsT streamed as 4 quarter-K tiles per s-chunk
    NOUT = (QC // P) * (H // QC)  # outproj tiles per chunk (16)
    NCH = B * nq      # total chunks (8)

    nc = bacc.Bacc("TRN2", target_bir_lowering=False, debug=False, num_devices=NCORES)
    hsT = nc.dram_tensor("hsT", [H, bs], PROJ_DT, kind="ExternalInput")
    maskT = nc.dram_tensor("maskT", [B, s, s], MASK_DT, kind="ExternalInput")
    wqT = nc.dram_tensor("wqT", [H, DC], PROJ_DT, kind="ExternalInput")
    wkT = nc.dram_tensor("wkT", [H, DC], PROJ_DT, kind="ExternalInput")
    wvT = nc.dram_tensor("wvT", [H, DC], PROJ_DT, kind="ExternalInput")
    woT = nc.dram_tensor("woT", [DC, H], OUT_DT, kind="ExternalInput")
    out = nc.dram_tensor("out", [bs, H], OUT_DT, kind="ExternalOutput")

    hsT_r = hsT.ap().rearrange("(o p) t -> p o t", p=P)
    wq_r = wqT.ap().rearrange("(o p) d -> p o d", p=P)
    wk_r = wkT.ap().rearrange("(o p) d -> p o d", p=P)
    wv_r = wvT.ap().rearrange("(o p) d -> p o d", p=P)
    wo_r = woT.ap().rearrange("(h p) j -> p h j", p=P)
    out_r = out.ap().rearrange("(t p) j -> p t j", p=P)

    with tile.TileContext(nc) as tc:
        with (
            tc.tile_pool(name="persist", bufs=1) as persist,
        ):
            qT = persist.tile([P, HPC, bs], QK_DT)
            kT = persist.tile([P, HPC, bs], QK_DT)
            vsb = persist.tile([P, vt_n, DC], BF16)
            wo_sb = persist.tile([P, HPC, H], OUT_DT)
            wq_sb = persist.tile([P, HT, DC], PROJ_DT, name="wq_sb")
            wk_sb = persist.tile([P, HT, DC], PROJ_DT, name="wk_sb")
            wv_sb = persist.tile([P, HT, DC], PROJ_DT, name="wv_sb")

            # evacuation helper: alternate DVE/ACT so neither paces the PE,
            # with an optional forced engine for load balancing
            _ev = [0]

            def evac(dst, src, eng=None):
                if eng is None:
                    eng = "s" if _ev[0] % 2 == 0 else "v"
                    _ev[0] += 1
                if eng == "s":
                    nc.scalar.copy(dst, src)
                else:
                    nc.vector.tensor_copy(dst, src)

            # hsT quarter tiles stream through a ring shared by both phases
            hpool_cm = tc.tile_pool(name="hpool", bufs=7)
            hpool = hpool_cm.__enter__()
            quarters = {}  # sc -> [4 quarter tiles]

            def hst_issue(sc, eng=None):
                eng = eng or nc.sync
                ssl = slice(sc * QC, (sc + 1) * QC)
                qs = []
                for qf in range(NQT):
                    hst = hpool.tile([P, KOQ, QC], PROJ_DT, tag="hst", name="hst")
                    eng.dma_start(
                        hst[:], hsT_r[:, qf * KOQ : (qf + 1) * KOQ, ssl]
                    )
                    qs.append(hst)
                quarters[sc] = qs

            def hq(sc, ko):
                return quarters[sc][ko // KOQ][:, ko % KOQ]

            # mask halves: ring of 3, issued one per period start
            mpool_cm = tc.tile_pool(name="mpool", bufs=3)
            mpool = mpool_cm.__enter__()
            mhalves = {}  # (chunk, mh) -> tile

            def mask_issue(ch, mh):
                if (ch, mh) in mhalves:
                    return
                b, qq = divmod(ch, nq)
                ms = mpool.tile([P, KH, QC], MASK_DT, tag="mslab", name="ms")
                nc.sync.dma_start(
                    ms[:],
                    maskT.ap()[b].rearrange("(kt p) q -> p kt q", p=P)[
                        :, mh * KH : (mh + 1) * KH,
                        qq * QC : (qq + 1) * QC,
                    ],
                )
                mhalves[(ch, mh)] = ms

            # ---------------- Phase A: batch-0 projections ----------------
            with (
                tc.tile_pool(name="ppsum", bufs=2, space="PSUM") as ppsum,
                tc.tile_pool(name="vpsum", bufs=4, space="PSUM") as vpsum,
            ):
                # DMA order: first q-projection group needs wq quarter 0 and
                # the first hsT quarter — issue those first.
                # issue the critical first tiles on TWO DGE queues in
                # parallel: wq quarter 0 on Sync, hsT chunk 0 on ScalarE
                # (idle at startup); the rest follows on Sync
                nc.sync.dma_start(wq_sb[:, 0:4], wq_r[:, 0:4])
                hst_issue(0, nc.scalar)
                for _wf in range(1, 4):
                    _wsl = slice(_wf * (HT // 4), (_wf + 1) * (HT // 4))
                    nc.sync.dma_start(wq_sb[:, _wsl], wq_r[:, _wsl])
                nc.sync.dma_start(wk_sb[:], wk_r)
                nc.sync.dma_start(wv_sb[:], wv_r)
                nc.sync.dma_start(wo_sb[:], wo_r)

                for sc in range(nq):  # batch 0 chunks only
                    if sc > 0:
                        hst_issue(sc)
                    ssl = slice(sc * QC, (sc + 1) * QC)
                    if sc == 2:
                        mask_issue(0, 0)
                        mask_issue(0, 1)
                    # q(h0), q(h1), k(h0), k(h1) — q first so the wk DMA has
                    # more slack at startup
                    for wsb, dstT in ((wq_sb, qT), (wk_sb, kT)):
                        for h in range(HPC):
                            ps = ppsum.tile([P, QC], F32, tag="psqk")
                            for ko in range(HT):
                                nc.tensor.matmul(
                                    ps[:],
                                    wsb[:, ko, h * P : (h + 1) * P],
                                    hq(sc, ko),
                                    start=(ko == 0),
                                    stop=(ko == HT - 1),
                                )
                            evac(dstT[:, h, ssl], ps[:])
                    # v: ko-outer over 4 concurrent PSUM groups
                    psvs = []
                    for st in range(QC // P):
                        psv = vpsum.tile([P, DC], F32, tag="psv")
                        psvs.append(psv)
                    for ko in range(HT):
                        for st in range(QC // P):
                            nc.tensor.matmul(
                                psvs[st][:],
                                hq(sc, ko)[:, st * P : (st + 1) * P],
                                wv_sb[:, ko, :],
                                start=(ko == 0),
                                stop=(ko == HT - 1),
                            )
                    for st in range(QC // P):
                        evac(vsb[:, sc * (QC // P) + st, :], psvs[st][:])
                # prefetch first b1 chunk's hsT for the phase-2 filler
                hst_issue(nq)

            # ---------------- Phase 2: merged attention + b1 projections ----------------
            with (
                tc.tile_pool(name="apool", bufs=2) as apool,
                tc.tile_pool(name="tpool", bufs=2) as tpool,
                tc.tile_pool(name="upool", bufs=2) as upool,
                tc.tile_pool(name="rpool", bufs=1) as rpool,
                tc.tile_pool(name="opool", bufs=6) as opool,
                tc.tile_pool(name="spsum", bufs=2, space="PSUM") as spsum,
                tc.tile_pool(name="upsum", bufs=2, space="PSUM") as upsum,
            ):
                # single-bank pools for periods 0-7 (outproj tiles + b1
                # projections), manually scoped: both release after period 7
                # and their two banks recycle as ONE two-bank outproj pool so
                # late-period tiles are drawn in adjacent-jc PAIRS with a
                # single [128,1024] evac + single DMA per pair
                opsum_cm = tc.tile_pool(name="opsum", bufs=1, space="PSUM")
                opsum = opsum_cm.__enter__()
                pproj_cm = tc.tile_pool(name="pproj", bufs=1, space="PSUM")
                pproj = pproj_cm.__enter__()
                odsum_holder = [None, None]  # (cm, pool)
                units = [
                    (b, qq, h)
                    for b in range(B)
                    for qq in range(nq)
                    for h in range(HPC)
                ]
                nu = len(units)
                aslabs = {}
                psus = {}
                s8s = {}
                rsums = {}
                rbcs = {}
                uTs = {}

                def qk_pair(i, j):
                    # two scoresT k-tile matmuls into one 2-bank psum tile;
                    # exp(s+m) = exp(s)*exp(m): ScalarE exp evacuates the
                    # [128,1024] pair in one ACT op; the host-precomputed
                    # exp(mask) factor is one fp16 [128,1024] DVE multiply
                    b, qq, h = units[i]
                    ch = i // HPC
                    if j == 0:
                        asl = apool.tile([P, kt_n, QC], BF16, tag="aslab", name="asl")
                        aslabs[i] = asl
                    asl = aslabs[i]
                    pss = spsum.tile([P, 2, QC], F32, tag="pss")
                    for u in range(2):
                        kt = 2 * j + u
                        nc.tensor.matmul(
                            pss[:, u],
                            kT[:, h, b * s + kt * P : b * s + (kt + 1) * P],
                            qT[:, h, b * s + qq * QC : b * s + (qq + 1) * QC],
                            start=True,
                            stop=True,
                        )
                    nc.scalar.activation(asl[:, 2 * j : 2 * j + 2], pss[:], EXP)
                    ms = mhalves[(ch, (2 * j) // KH)]
                    mo = (2 * j) % KH
                    nc.vector.tensor_tensor(
                        asl[:, 2 * j : 2 * j + 2],
                        asl[:, 2 * j : 2 * j + 2],
                        ms[:, mo : mo + 2],
                        MULT,
                    )

                def pv_part(i, kt):
                    b, qq, h = units[i]
                    asl = aslabs[i]
                    if kt == 0:
                        psu = upsum.tile([P, QC], F32, tag="psu")
                        psus[i] = psu
                    nc.tensor.matmul(
                        psus[i][:],
                        vsb[:, b * kt_n + kt, h * P : (h + 1) * P],
                        asl[:, kt],
                        start=(kt == 0),
                        stop=(kt == kt_n - 1),
                    )

                def den_tree_level(i, lvl):
                    # kt-fold of the attn slab into a scratch tile (asl is
                    # fully written at period start); fp16 2x DVE adds
                    asl = aslabs[i]
                    if lvl == 0:
                        s8 = tpool.tile([P, KH, QC], BF16, tag="s8", name="s8")
                        s8s[i] = s8
                        nc.vector.tensor_tensor(
                            s8[:], asl[:, 0:KH], asl[:, KH : 2 * KH], ADD
                        )
                    else:
                        s8 = s8s[i]
                        w = KH >> lvl
                        nc.vector.tensor_tensor(
                            s8[:, 0:w], s8[:, 0:w], s8[:, w : 2 * w], ADD
                        )

                def den_all_reduce(i):
                    # ONE GpSimd partition_all_reduce turns the folded
                    # [128(k), 512(q)] tile into broadcast row sums [128, 512]
                    rsum = rpool.tile([P, QC], F32, tag="rsum", name="rsum")
                    nc.gpsimd.partition_all_reduce(
                        rsum[:], s8s.pop(i)[:, 0], 128, bass_isa.ReduceOp.add
                    )
                    rsums[i] = rsum

                def den_recip(i):
                    # in-place reciprocal: rbc aliases the all-reduce output
                    rbc = rsums.pop(i)
                    nc.vector.reciprocal_approx_fast(out=rbc[:], in_=rbc[:])
                    rbcs[i] = rbc

                def uT_mult(i):
                    # normalize + evacuate the PV accumulator in one DVE op
                    b, qq, h = units[i]
                    if b not in uTs:
                        uT_new = upool.tile([P, HPC, s], OUT_DT, tag="uT", name="uT")
                        uTs[b] = uT_new
                    nc.vector.tensor_tensor(
                        uTs[b][:, h, qq * QC : (qq + 1) * QC],
                        psus.pop(i)[:],
                        rbcs.pop(i)[:],
                        MULT,
                    )

                _ot_n = [0]

                def outproj_tile(b, qq, t, pso=None, eng=None):
                    # one output-projection tile: 2 accumulating matmuls
                    # (h=0,1) + evac + DMA out; late periods alternate two
                    # independent banks so the PE never waits an evac
                    st_l, jc = divmod(t, H // QC)
                    st = qq * (QC // P) + st_l
                    uT_b = uTs[b]
                    if pso is None:
                        o2 = odsum_holder[1]
                        if o2 is not None and _ot_n[0] % 2 == 1:
                            pso = o2.tile([P, QC], F32, tag="pso2", name="pso2")
                        else:
                            pso = opsum.tile([P, QC], F32, tag="pso", name="pso")
                        _ot_n[0] += 1
                    for h in range(HPC):
                        nc.tensor.matmul(
                            pso[:],
                            uT_b[:, h, st * P : (st + 1) * P],
                            wo_sb[:, h, jc * QC : (jc + 1) * QC],
                            start=(h == 0),
                            stop=(h == HPC - 1),
                        )
                    ot = opool.tile([P, QC], OUT_DT, tag="ot", name="ot", bufs=4)
                    evac(ot[:], pso[:], eng=eng)
                    nc.sync.dma_start(
                        out_r[:, b * st_n + st, jc * QC : (jc + 1) * QC], ot[:]
                    )

                def outproj_pair(b, qq, t0, pso2, eng=None):
                    # two adjacent-jc tiles into a 2-bank psum tile: 4 MMs,
                    # ONE [128,1024] evac, ONE contiguous DMA (drain only)
                    st_l, jc = divmod(t0, H // QC)
                    st = qq * (QC // P) + st_l
                    uT_b = uTs[b]
                    for u in range(2):
                        for h in range(HPC):
                            nc.tensor.matmul(
                                pso2[:, u],
                                uT_b[:, h, st * P : (st + 1) * P],
                                wo_sb[:, h, (jc + u) * QC : (jc + u + 1) * QC],
                                start=(h == 0),
                                stop=(h == HPC - 1),
                            )
                    ot = opool.tile([P, 2, QC], OUT_DT, tag="ot2", name="ot2", bufs=2)
                    evac(ot[:], pso2[:], eng=eng)
                    nc.sync.dma_start(
                        out_r[:, b * st_n + st, jc * QC : (jc + 2) * QC], ot[:]
                    )

                # ---- b1 projection filler groups (shared single PSUM bank)
                def proj_group_qk(wsb, dstT, h, sc):
                    ps = pproj.tile([P, QC], F32, tag="pj", name="pj")
                    ssl = slice(sc * QC, (sc + 1) * QC)
                    for ko in range(HT):
                        nc.tensor.matmul(
                            ps[:],
                            wsb[:, ko, h * P : (h + 1) * P],
                            hq(sc, ko),
                            start=(ko == 0),
                            stop=(ko == HT - 1),
                        )
                    evac(dstT[:, h, ssl], ps[:])

                def proj_group_v(st, sc):
                    psv = pproj.tile([P, DC], F32, tag="pj", name="pj")
                    for ko in range(HT):
                        nc.tensor.matmul(
                            psv[:],
                            hq(sc, ko)[:, st * P : (st + 1) * P],
                            wv_sb[:, ko, :],
                            start=(ko == 0),
                            stop=(ko == HT - 1),
                        )
                    evac(vsb[:, sc * (QC // P) + st, :], psv[:])

                proj_jobs = []
                for sc in range(nq, 2 * nq):  # b1 global chunks 4..7
                    proj_jobs.append(("qk", wq_sb, qT, 0, sc))
                    proj_jobs.append(("qk", wq_sb, qT, 1, sc))
                    proj_jobs.append(("qk", wk_sb, kT, 0, sc))
                    proj_jobs.append(("qk", wk_sb, kT, 1, sc))
                    proj_jobs.append(("v", None, None, 0, sc))
                    proj_jobs.append(("v", None, None, 1, sc))
                    proj_jobs.append(("v", None, None, 2, sc))
                    proj_jobs.append(("v", None, None, 3, sc))
                _pj = [0]

                def emit_proj_job():
                    if _pj[0] >= len(proj_jobs):
                        return
                    kind, wsb, dstT, x, sc = proj_jobs[_pj[0]]
                    _pj[0] += 1
                    if kind == "qk":
                        proj_group_qk(wsb, dstT, x, sc)
                    else:
                        proj_group_v(x, sc)
                    # prefetch the next b1 chunk's hsT once this chunk's last
                    # group is emitted
                    if _pj[0] % 8 == 0 and sc + 1 < 2 * nq:
                        hst_issue(sc + 1)

                # outproj backlog pacing: uniform 8/period (availability
                # capped: chunk c's 16 tiles appear at period 2c+2). Early
                # periods draw singles at even j (projection groups own odd
                # j); late periods draw PAIRS at odd j.
                DRAW = [0, 0] + [8] * 14
                JD_EARLY = [0, 0, 2, 0, 2, 0, 2, 2]
                JD_LATE = [1, 1, 1, 1, 1, 1, 1, 1]
                backlog = []
                # outproj evac engine pattern: biased to ScalarE (its exp
                # stream leaves it more slack than DVE's mask+tree+uT+recip)
                EVPAT = ("s", "s", "v")

                # ---- pipeline prologue: unit 0's QK pairs
                for j in range(kp_n):
                    qk_pair(0, j)

                for i in range(nu):
                    b, qq, h = units[i]
                    if i == 8:
                        # projections done: recycle the pproj bank as a
                        # second outproj bank (alternating with opsum)
                        pproj_cm.__exit__(None, None, None)
                        cm = tc.tile_pool(name="odsum", bufs=1, space="PSUM")
                        odsum_holder[0] = cm
                        odsum_holder[1] = cm.__enter__()
                    # mask prefetch: chunk i//2+1, half i%2
                    ch_next = i // 2 + 1
                    if ch_next < NCH:
                        mask_issue(ch_next, i % 2)
                    # denominator chain of i-1 + uT evac (rbc long ready)
                    if i >= 1:
                        den_recip(i - 1)
                        uT_mult(i - 1)
                        if units[i - 1][2] == HPC - 1:
                            pc = (i - 1) // 2
                            pb, pqq = pc // nq, pc % nq
                            backlog.extend((pb, pqq, t) for t in range(NOUT))
                    jdist = JD_EARLY if i < 8 else JD_LATE
                    ndraw = 0
                    for j in range(kp_n):
                        if i + 1 < nu:
                            qk_pair(i + 1, j)
                        pv_part(i, 2 * j)
                        pv_part(i, 2 * j + 1)
                        if j <= 3:
                            den_tree_level(i, j)
                        if j == 3:
                            den_all_reduce(i)
                        for _ in range(jdist[j]):
                            if backlog and ndraw < DRAW[i]:
                                pb, pqq, t = backlog.pop(0)
                                outproj_tile(
                                    pb, pqq, t, eng=EVPAT[ndraw % len(EVPAT)]
                                )
                                ndraw += 1
                        if i < 8 and j % 2 == 1:
                            emit_proj_job()
                # drain: last unit's den chain + final chunk's outproj with
                # deep psum pipelining (borrow the freed QK pair banks)
                den_recip(nu - 1)
                uT_mult(nu - 1)
                rest = list(backlog) + [(B - 1, nq - 1, t) for t in range(NOUT)]
                for n in range(0, len(rest), 2):
                    ob, oqq, t0 = rest[n]
                    pso2 = spsum.tile([P, 2, QC], F32, tag="pss", name="big")
                    outproj_pair(ob, oqq, t0, pso2=pso2)
                odsum_holder[0].__exit__(None, None, None)
                opsum_cm.__exit__(None, None, None)

            mpool_cm.__exit__(None, None, None)
            hpool_cm.__exit__(None, None, None)
    nc.compile()
    return nc


def make_in_maps(hs, mask, Wq, Wk, Wv, Wo):
    """Host-side prep: transpose/shard the full inputs into per-core maps."""
    bs = hs.shape[0] * hs.shape[1]
    proj_np = np.float16
    out_np = np.float16
    hsT = np.ascontiguousarray(hs.reshape(bs, H).T).astype(proj_np)
    maskT = np.exp(
        np.ascontiguousarray(mask[:, 0].transpose(0, 2, 1))
    ).astype(np.float16)
    in_maps = []
    for c in range(NCORES):
        sl = slice(c * DC, (c + 1) * DC)
        in_maps.append(
            {
                "hsT": hsT,
                "maskT": maskT,
                "wqT": np.ascontiguousarray((Wq[sl] * SCALE).T).astype(proj_np),
                "wkT": np.ascontiguousarray(Wk[sl].T).astype(proj_np),
                "wvT": np.ascontiguousarray(Wv[sl].T).astype(proj_np),
                "woT": np.ascontiguousarray(Wo[:, sl].T).astype(out_np),
            }
        )
    return in_maps


_NC_CACHE = {}


def get_nc(s=S):
    if s not in _NC_CACHE:
        _NC_CACHE[s] = build_attention_nc(s)
    return _NC_CACHE[s]


def run(hs, mask, Wq, Wk, Wv, Wo, trace=False, trace_kwargs=None):
    s = hs.shape[1]
    nc = get_nc(s)
    in_maps = make_in_maps(hs, mask, Wq, Wk, Wv, Wo)
    res = run_bass_kernel_spmd(
        nc,
        in_maps,
        core_ids=list(range(NCORES)),
        trace=trace,
        **(trace_kwargs or {}),
    )
    parts = np.stack([r["out"] for r in res.results])
    full = parts.astype(np.float32).sum(axis=0)
    return full.reshape(hs.shape[0], s, H), res


def kernel(hidden_states, attention_mask, Wq, Wk, Wv, Wo):
    hs = np.asarray(hidden_states, dtype=np.float32)
    mask = np.asarray(attention_mask, dtype=np.float32)
    Wq = np.asarray(Wq, dtype=np.float32)
    Wk = np.asarray(Wk, dtype=np.float32)
    Wv = np.asarray(Wv, dtype=np.float32)
    Wo = np.asarray(Wo, dtype=np.float32)
    out, _ = run(hs, mask, Wq, Wk, Wv, Wo)
    return out
